# revision 17
# baseline (speedup 1.0000x reference)
"""Trainium2 Bass kernel for nn_HardMemory (retrieval_knn).

For each spatial token (B*H*W tokens, C=128 channels), find the memory row
(of M=512) with max cosine similarity and replace the token's channel vector
with that raw memory row.

Algebraic simplification: argmax_m cos(x, mem_m) = argmax_m (x . mem_n_m)
where mem_n is the l2-normalized memory -- normalizing x is a positive
per-token scale and cannot change the argmax, so it is skipped.

Scores (scaled by S=512 so the fp8 correction terms are representable):
  ps = (S*xh16) @ mh16  +  DoubleRow{ xl8 @ mh8  +  xh8 @ ml8 }
where xh16 = fp16(x), xl8 = e4m3(S*(x - xh16)), xh8 = e4m3(xh16),
mh16 = fp16(mem_n^T), mh8 = e4m3(mh16), ml8 = e4m3(S*(mem_n^T - mh16)).
The fp8 DoubleRow matmul computes both correction terms in one pass at
0.5 cycles/col. Measured on the fixed seed: 1 argmax flip vs fp64
(output rel err 4e-3, gate is 2e-2).

Per-core pipeline (data-parallel over batch, 4 batches/core), groups of
2 tiles (256 tokens):
  PE  : scores into PSUM [128tok, 2, 512] fp32 (2 matmuls per tile)
  ACT : copy PSUM -> SBUF fp32 (s32), batched
  POOL: mx = reduce_max(s32) [128, 2]
  DVE : oh16 = (s32 >= mx) per tile (2x mode, all-SBUF)
  PE  : 4x transpose oh16 chunk -> ohT PSUM fp16
  DVE : copy ohT PSUM -> SBUF fp16 (2x mode)
  PE  : gather: out[c, tok] = sum_k mem16_k^T @ ohT_k (4 fp16 matmuls)
  ACT/DVE: copy gather PSUM -> out16 SBUF fp16; DMA out per 8 tiles
Output returned as fp16 rows upcast to fp32 on host (mem quantization
rel err ~1e-4, well under the gate).
"""

import numpy as np

import concourse.bass as bass
import concourse.mybir as mybir
from concourse.tile import TileContext
from concourse.bass_utils import run_bass_kernel_spmd

F32 = mybir.dt.float32
F16 = mybir.dt.float16
F8 = mybir.dt.float8e4

B, C, H, W = 32, 128, 64, 64
N = H * W              # 4096 tokens per batch
M = 512                # memory rows
NCORES = 8
BPC = B // NCORES      # batches per core
TOK = BPC * N          # tokens per core (16384)
TILE = 128             # tokens per tile
NTILES = TOK // TILE   # 128
TPB = N // TILE        # tiles per batch (32)
GRP = 2                # tiles per score group (PSUM budget)
OUTG = 8               # tiles per output DMA chunk (1024 tokens)
KCH = M // TILE        # 4 gather chunks
S_SCALE = 512.0


def _build():
    nc = bass.Bass(trn_type="TRN2")

    xs_in = nc.dram_tensor("xs", [BPC, C, N], F16, kind="ExternalInput")
    x8_in = nc.dram_tensor("x8", [BPC, C, TPB, 2, TILE], F8, kind="ExternalInput")
    mh_in = nc.dram_tensor("mh", [C, M], F16, kind="ExternalInput")
    m8_in = nc.dram_tensor("m8", [C, 2, M], F8, kind="ExternalInput")
    gm_in = nc.dram_tensor("gm", [TILE, KCH, C], F16, kind="ExternalInput")
    ident_in = nc.dram_tensor("ident", [TILE, TILE], F16, kind="ExternalInput")
    out_d = nc.dram_tensor("out", [C, TOK], F16, kind="ExternalOutput")

    with TileContext(nc) as tc:
        with (
            tc.tile_pool(name="const", bufs=1) as cpool,
            tc.tile_pool(name="xin", bufs=3) as xpool,
            tc.tile_pool(name="s32", bufs=5) as spool,
            tc.tile_pool(name="mx", bufs=8) as mxpool,
            tc.tile_pool(name="oh", bufs=5) as ohpool,
            tc.tile_pool(name="ohts", bufs=4) as ohtspool,
            tc.tile_pool(name="osb", bufs=3) as opool,
            tc.tile_pool(name="ps_s", bufs=2, space="PSUM") as ps_s,
            tc.tile_pool(name="ps_t", bufs=3, space="PSUM") as ps_t,
            tc.tile_pool(name="ps_o", bufs=1, space="PSUM") as ps_o,
        ):
            mh = cpool.tile([C, M], F16)
            nc.sync.dma_start(out=mh, in_=mh_in[:])
            m8 = cpool.tile([C, 2, M], F8)
            nc.sync.dma_start(out=m8, in_=m8_in[:])
            gm = cpool.tile([TILE, KCH, C], F16)
            nc.sync.dma_start(out=gm, in_=gm_in[:])
            ident = cpool.tile([TILE, TILE], F16)
            nc.sync.dma_start(out=ident, in_=ident_in[:])

            xs_sb = x8_sb = None
            ob = po = None
            NGRP = NTILES // GRP
            for g in range(NGRP):
                t = g * GRP
                b, tb0 = divmod(t, TPB)
                if tb0 == 0:
                    xs_sb = xpool.tile([C, N], F16, tag="xs")
                    nc.sync.dma_start(out=xs_sb, in_=xs_in[b, :, :])
                    x8_sb = xpool.tile([C, TPB, 2, TILE], F8, tag="x8")
                    nc.sync.dma_start(out=x8_sb, in_=x8_in[b, :, :, :, :])

                ps = ps_s.tile([TILE, GRP, M], F32)
                for q in range(GRP):
                    tb = tb0 + q
                    nc.tensor.matmul(
                        out=ps[:, q, :],
                        lhsT=xs_sb[:, tb * TILE : (tb + 1) * TILE],
                        rhs=mh,
                        start=True,
                        stop=False,
                    )
                    nc.tensor.matmul(
                        out=ps[:, q, :],
                        lhsT=x8_sb[:, tb, :, :],
                        rhs=m8,
                        start=False,
                        stop=True,
                        perf_mode=mybir.MatmulPerfMode.DoubleRow,
                    )

                s32 = spool.tile([TILE, GRP, M], F32)
                nc.scalar.activation(
                    out=s32, in_=ps, func=mybir.ActivationFunctionType.Copy
                )
                mx = mxpool.tile([TILE, GRP], F32)
                nc.vector.reduce_max(out=mx, in_=s32, axis=mybir.AxisListType.X)

                # one-hot on POOL (only SBUF-capable engine stage)
                oh = ohpool.tile([TILE, GRP, M], F16)
                for q in range(GRP):
                    nc.gpsimd.tensor_scalar(
                        out=oh[:, q, :], in0=s32[:, q, :],
                        scalar1=mx[:, q : q + 1], scalar2=None,
                        op0=mybir.AluOpType.is_ge,
                    )

                oht_ps = ps_t.tile([TILE, KCH, GRP * TILE], F16)
                for q in range(GRP):
                    for k in range(KCH):
                        nc.tensor.transpose(
                            out=oht_ps[:, k, q * TILE : (q + 1) * TILE],
                            in_=oh[:, q, k * TILE : (k + 1) * TILE],
                            identity=ident,
                        )
                oht = ohtspool.tile([TILE, KCH, GRP * TILE], F16)
                # ohT copy: ~70% DVE (fp16 2x), ~30% ACT
                if g % 10 >= 7:
                    nc.scalar.activation(
                        out=oht, in_=oht_ps,
                        func=mybir.ActivationFunctionType.Copy,
                    )
                else:
                    nc.vector.tensor_copy(out=oht, in_=oht_ps)

                if g % 2 == 0:
                    po = ps_o.tile([C, 2 * GRP * TILE], F32)
                half = (g % 2) * GRP * TILE
                for k in range(KCH):
                    nc.tensor.matmul(
                        out=po[:, half : half + GRP * TILE],
                        lhsT=gm[:, k, :],
                        rhs=oht[:, k, :],
                        start=(k == 0),
                        stop=(k == KCH - 1),
                    )
                if g % 2 == 1:
                    # batched out copy of two groups (512 tokens) on ACT
                    oslot = ((g - 1) * GRP * TILE) % (OUTG * TILE)
                    if oslot == 0:
                        ob = opool.tile([C, OUTG * TILE], F16)
                    nc.scalar.activation(
                        out=ob[:, oslot : oslot + 2 * GRP * TILE],
                        in_=po, func=mybir.ActivationFunctionType.Copy,
                    )
                    if oslot + 2 * GRP * TILE == OUTG * TILE:
                        tok0 = (g + 1) * GRP * TILE - OUTG * TILE
                        nc.sync.dma_start(
                            out=out_d[:, tok0 : tok0 + OUTG * TILE], in_=ob
                        )

    _legalize_waits(nc)
    nc.finalize()
    return nc


def _legalize_waits(nc):
    """This container's walrus accepts only ONE sync wait per engine
    instruction (setupSyncWait: 'Too many sync wait commands'). Tile emits
    multi-wait instructions (and a multi-wait tail drain). Split: keep one
    wait on the instruction, hoist the rest onto single-wait Drain ops
    inserted just before it on the same engine (engine order preserved =>
    semantics preserved). DMA copies are left alone (ring descriptors
    accept multiple waits)."""
    n_split = 0
    for f in nc.m.functions:
        for b in f.blocks:
            out = []
            for inst in b.instructions:
                si = inst.sync_info
                if si is not None and len(si.on_wait) > 1:
                    waits = list(si.on_wait)
                    for j, w in enumerate(waits[:-1]):
                        out.append(
                            mybir.InstDrain(
                                name=f"{inst.name}-w{j}",
                                engine=inst.engine,
                                ins=[],
                                outs=[],
                                sync_info=mybir.SyncInfo(
                                    on_wait=[w], on_update=[]
                                ),
                            )
                        )
                    inst.sync_info = mybir.SyncInfo(
                        on_wait=[waits[-1]], on_update=list(si.on_update)
                    )
                    n_split += 1
                out.append(inst)
            b.instructions = out
    return n_split


_NC = None


def _get_nc():
    global _NC
    if _NC is None:
        _NC = _build()
    return _NC


def _host_prep(x, memory):
    f8 = mybir.dt.np(F8)
    memn = memory / np.maximum(
        np.sqrt((memory * memory).sum(axis=1, keepdims=True)), 1e-12
    )
    mnt = np.ascontiguousarray(memn.T).astype(np.float32)          # [C, M]
    mh = mnt.astype(np.float16)                                     # fp16 main
    ml = (mnt - mh.astype(np.float32)) * S_SCALE                    # residual
    m8 = np.empty((C, 2, M), dtype=f8)
    m8[:, 0, :] = mh.astype(f8)          # pairs with xl8
    m8[:, 1, :] = ml.astype(f8)          # pairs with xh8

    # raw memory rows fp16 for the gather: gm[p, k, c] = mem[k*128+p, c]
    gm = np.ascontiguousarray(
        memory.astype(np.float16).reshape(KCH, TILE, C).transpose(1, 0, 2)
    )

    xh = x.astype(np.float16)                                       # [B,C,H,W]
    xs = (xh.astype(np.float32) * S_SCALE).astype(np.float16)       # exact
    xl = (x - xh.astype(np.float32)) * S_SCALE
    # x8 packed per batch: [C, TPB, 2, TILE]; plane0=xl8, plane1=xh8
    xl8 = xl.reshape(B, C, TPB, TILE).astype(f8)
    xh8 = xh.reshape(B, C, TPB, TILE).astype(f8)
    x8 = np.stack([xl8, xh8], axis=3)                               # [B,C,TPB,2,TILE]

    ident = np.eye(TILE, dtype=np.float16)
    return xs.reshape(B, C, N), x8, mh, m8, gm, ident


def kernel(x, memory):
    x = np.asarray(x, dtype=np.float32)
    memory = np.asarray(memory, dtype=np.float32)
    nc = _get_nc()
    xs, x8, mh, m8, gm, ident = _host_prep(x, memory)

    in_maps = []
    for c in range(NCORES):
        sl = slice(c * BPC, (c + 1) * BPC)
        in_maps.append({
            "xs": np.ascontiguousarray(xs[sl]),
            "x8": np.ascontiguousarray(x8[sl]),
            "mh": mh, "m8": m8, "gm": gm, "ident": ident,
        })

    res = run_bass_kernel_spmd(nc, in_maps, core_ids=list(range(NCORES)))
    # out per core: [C, TOK] fp16, tokens in natural order (b*N + n)
    outs = []
    for r in res.results:
        oc = r["out"].astype(np.float32).reshape(C, BPC, N)
        outs.append(oc.transpose(1, 0, 2).reshape(BPC, C, H, W))
    return np.concatenate(outs, axis=0)


# revision 18
# speedup vs baseline: 1.0520x; 1.0520x over previous
"""Trainium2 Bass kernel for nn_HardMemory (retrieval_knn).

For each spatial token (B*H*W tokens, C=128 channels), find the memory row
(of M=512) with max cosine similarity and replace the token's channel vector
with that raw memory row.

Algebraic simplification: argmax_m cos(x, mem_m) = argmax_m (x . mem_n_m)
where mem_n is the l2-normalized memory -- normalizing x is a positive
per-token scale and cannot change the argmax, so it is skipped.

Scores (scaled by S=512 so the fp8 correction terms are representable):
  ps = (S*xh16) @ mh16  +  DoubleRow{ xl8 @ mh8  +  xh8 @ ml8 }
where xh16 = fp16(x), xl8 = e4m3(S*(x - xh16)), xh8 = e4m3(xh16),
mh16 = fp16(mem_n^T), mh8 = e4m3(mh16), ml8 = e4m3(S*(mem_n^T - mh16)).
The fp8 DoubleRow matmul computes both correction terms in one pass at
0.5 cycles/col. Measured on the fixed seed: 1 argmax flip vs fp64
(output rel err 4e-3, gate is 2e-2).

Per-core pipeline (data-parallel over batch, 4 batches/core), groups of
2 tiles (256 tokens):
  PE  : scores into PSUM [128tok, 2, 512] fp32 (2 matmuls per tile)
  ACT : copy PSUM -> SBUF fp32 (s32), batched
  POOL: mx = reduce_max(s32) [128, 2]
  DVE : oh16 = (s32 >= mx) per tile (2x mode, all-SBUF)
  PE  : 4x transpose oh16 chunk -> ohT PSUM fp16
  DVE : copy ohT PSUM -> SBUF fp16 (2x mode)
  PE  : gather: out[c, tok] = sum_k mem16_k^T @ ohT_k (4 fp16 matmuls)
  ACT/DVE: copy gather PSUM -> out16 SBUF fp16; DMA out per 8 tiles
Output returned as fp16 rows upcast to fp32 on host (mem quantization
rel err ~1e-4, well under the gate).
"""

import numpy as np

import concourse.bass as bass
import concourse.mybir as mybir
from concourse.tile import TileContext
from concourse.bass_utils import run_bass_kernel_spmd

F32 = mybir.dt.float32
F16 = mybir.dt.float16
F8 = mybir.dt.float8e4

B, C, H, W = 32, 128, 64, 64
N = H * W              # 4096 tokens per batch
M = 512                # memory rows
NCORES = 8
BPC = B // NCORES      # batches per core
TOK = BPC * N          # tokens per core (16384)
TILE = 128             # tokens per tile
NTILES = TOK // TILE   # 128
TPB = N // TILE        # tiles per batch (32)
GRP = 2                # tiles per score group (PSUM budget)
OUTG = 8               # tiles per output DMA chunk (1024 tokens)
KCH = M // TILE        # 4 gather chunks
S_SCALE = 512.0


def _build():
    nc = bass.Bass(trn_type="TRN2")

    xs_in = nc.dram_tensor("xs", [BPC, C, N], F16, kind="ExternalInput")
    x8_in = nc.dram_tensor("x8", [BPC, C, TPB, 2, TILE], F8, kind="ExternalInput")
    mh_in = nc.dram_tensor("mh", [C, M], F16, kind="ExternalInput")
    m8_in = nc.dram_tensor("m8", [C, 2, M], F8, kind="ExternalInput")
    gm_in = nc.dram_tensor("gm", [TILE, KCH, C], F16, kind="ExternalInput")
    ident_in = nc.dram_tensor("ident", [TILE, TILE], F16, kind="ExternalInput")
    out_d = nc.dram_tensor("out", [C, TOK], F16, kind="ExternalOutput")

    with TileContext(nc) as tc:
        with (
            tc.tile_pool(name="const", bufs=1) as cpool,
            tc.tile_pool(name="xin", bufs=3) as xpool,
            tc.tile_pool(name="s32", bufs=5) as spool,
            tc.tile_pool(name="mx", bufs=8) as mxpool,
            tc.tile_pool(name="oh", bufs=5) as ohpool,
            tc.tile_pool(name="ohts", bufs=4) as ohtspool,
            tc.tile_pool(name="osb", bufs=3) as opool,
            tc.tile_pool(name="ps_s", bufs=2, space="PSUM") as ps_s,
            tc.tile_pool(name="ps_t", bufs=2, space="PSUM") as ps_t,
            tc.tile_pool(name="ps_o", bufs=2, space="PSUM") as ps_o,
        ):
            mh = cpool.tile([C, M], F16)
            nc.sync.dma_start(out=mh, in_=mh_in[:])
            m8 = cpool.tile([C, 2, M], F8)
            nc.sync.dma_start(out=m8, in_=m8_in[:])
            gm = cpool.tile([TILE, KCH, C], F16)
            nc.sync.dma_start(out=gm, in_=gm_in[:])
            ident = cpool.tile([TILE, TILE], F16)
            nc.sync.dma_start(out=ident, in_=ident_in[:])

            xs_sb = x8_sb = None
            ob = po = None
            NGRP = NTILES // GRP
            for g in range(NGRP):
                t = g * GRP
                b, tb0 = divmod(t, TPB)
                if tb0 == 0:
                    xs_sb = xpool.tile([C, N], F16, tag="xs")
                    nc.sync.dma_start(out=xs_sb, in_=xs_in[b, :, :])
                    x8_sb = xpool.tile([C, TPB, 2, TILE], F8, tag="x8")
                    nc.sync.dma_start(out=x8_sb, in_=x8_in[b, :, :, :, :])

                ps = ps_s.tile([TILE, GRP, M], F32)
                for q in range(GRP):
                    tb = tb0 + q
                    nc.tensor.matmul(
                        out=ps[:, q, :],
                        lhsT=xs_sb[:, tb * TILE : (tb + 1) * TILE],
                        rhs=mh,
                        start=True,
                        stop=False,
                    )
                    nc.tensor.matmul(
                        out=ps[:, q, :],
                        lhsT=x8_sb[:, tb, :, :],
                        rhs=m8,
                        start=False,
                        stop=True,
                        perf_mode=mybir.MatmulPerfMode.DoubleRow,
                    )

                s32 = spool.tile([TILE, GRP, M], F32)
                nc.scalar.activation(
                    out=s32, in_=ps, func=mybir.ActivationFunctionType.Copy
                )
                mx = mxpool.tile([TILE, GRP], F32)
                nc.vector.reduce_max(out=mx, in_=s32, axis=mybir.AxisListType.X)

                # one-hot on POOL (only SBUF-capable engine stage)
                oh = ohpool.tile([TILE, GRP, M], F16)
                for q in range(GRP):
                    nc.gpsimd.tensor_scalar(
                        out=oh[:, q, :], in0=s32[:, q, :],
                        scalar1=mx[:, q : q + 1], scalar2=None,
                        op0=mybir.AluOpType.is_ge,
                    )

                oht_ps = ps_t.tile([TILE, KCH, GRP * TILE], F16)
                for q in range(GRP):
                    for k in range(KCH):
                        nc.tensor.transpose(
                            out=oht_ps[:, k, q * TILE : (q + 1) * TILE],
                            in_=oh[:, q, k * TILE : (k + 1) * TILE],
                            identity=ident,
                        )
                oht = ohtspool.tile([TILE, KCH, GRP * TILE], F16)
                # ohT copy: ~70% DVE (fp16 2x), ~30% ACT
                if g % 10 >= 7:
                    nc.scalar.activation(
                        out=oht, in_=oht_ps,
                        func=mybir.ActivationFunctionType.Copy,
                    )
                else:
                    nc.vector.tensor_copy(out=oht, in_=oht_ps)

                if g % 2 == 0:
                    po = ps_o.tile([C, 2 * GRP * TILE], F32)
                half = (g % 2) * GRP * TILE
                for k in range(KCH):
                    nc.tensor.matmul(
                        out=po[:, half : half + GRP * TILE],
                        lhsT=gm[:, k, :],
                        rhs=oht[:, k, :],
                        start=(k == 0),
                        stop=(k == KCH - 1),
                    )
                if g % 2 == 1:
                    # batched out copy of two groups (512 tokens) on ACT
                    oslot = ((g - 1) * GRP * TILE) % (OUTG * TILE)
                    if oslot == 0:
                        ob = opool.tile([C, OUTG * TILE], F16)
                    nc.scalar.activation(
                        out=ob[:, oslot : oslot + 2 * GRP * TILE],
                        in_=po, func=mybir.ActivationFunctionType.Copy,
                    )
                    if oslot + 2 * GRP * TILE == OUTG * TILE:
                        tok0 = (g + 1) * GRP * TILE - OUTG * TILE
                        nc.sync.dma_start(
                            out=out_d[:, tok0 : tok0 + OUTG * TILE], in_=ob
                        )

    _legalize_waits(nc)
    nc.finalize()
    return nc


def _legalize_waits(nc):
    """This container's walrus accepts only ONE sync wait per engine
    instruction (setupSyncWait: 'Too many sync wait commands'). Tile emits
    multi-wait instructions (and a multi-wait tail drain). Split: keep one
    wait on the instruction, hoist the rest onto single-wait Drain ops
    inserted just before it on the same engine (engine order preserved =>
    semantics preserved). DMA copies are left alone (ring descriptors
    accept multiple waits)."""
    n_split = 0
    for f in nc.m.functions:
        for b in f.blocks:
            out = []
            for inst in b.instructions:
                si = inst.sync_info
                if si is not None and len(si.on_wait) > 1:
                    waits = list(si.on_wait)
                    for j, w in enumerate(waits[:-1]):
                        out.append(
                            mybir.InstDrain(
                                name=f"{inst.name}-w{j}",
                                engine=inst.engine,
                                ins=[],
                                outs=[],
                                sync_info=mybir.SyncInfo(
                                    on_wait=[w], on_update=[]
                                ),
                            )
                        )
                    inst.sync_info = mybir.SyncInfo(
                        on_wait=[waits[-1]], on_update=list(si.on_update)
                    )
                    n_split += 1
                out.append(inst)
            b.instructions = out
    return n_split


_NC = None


def _get_nc():
    global _NC
    if _NC is None:
        _NC = _build()
    return _NC


def _host_prep(x, memory):
    f8 = mybir.dt.np(F8)
    memn = memory / np.maximum(
        np.sqrt((memory * memory).sum(axis=1, keepdims=True)), 1e-12
    )
    mnt = np.ascontiguousarray(memn.T).astype(np.float32)          # [C, M]
    mh = mnt.astype(np.float16)                                     # fp16 main
    ml = (mnt - mh.astype(np.float32)) * S_SCALE                    # residual
    m8 = np.empty((C, 2, M), dtype=f8)
    m8[:, 0, :] = mh.astype(f8)          # pairs with xl8
    m8[:, 1, :] = ml.astype(f8)          # pairs with xh8

    # raw memory rows fp16 for the gather: gm[p, k, c] = mem[k*128+p, c]
    gm = np.ascontiguousarray(
        memory.astype(np.float16).reshape(KCH, TILE, C).transpose(1, 0, 2)
    )

    xh = x.astype(np.float16)                                       # [B,C,H,W]
    xs = (xh.astype(np.float32) * S_SCALE).astype(np.float16)       # exact
    xl = (x - xh.astype(np.float32)) * S_SCALE
    # x8 packed per batch: [C, TPB, 2, TILE]; plane0=xl8, plane1=xh8
    xl8 = xl.reshape(B, C, TPB, TILE).astype(f8)
    xh8 = xh.reshape(B, C, TPB, TILE).astype(f8)
    x8 = np.stack([xl8, xh8], axis=3)                               # [B,C,TPB,2,TILE]

    ident = np.eye(TILE, dtype=np.float16)
    return xs.reshape(B, C, N), x8, mh, m8, gm, ident


def kernel(x, memory):
    x = np.asarray(x, dtype=np.float32)
    memory = np.asarray(memory, dtype=np.float32)
    nc = _get_nc()
    xs, x8, mh, m8, gm, ident = _host_prep(x, memory)

    in_maps = []
    for c in range(NCORES):
        sl = slice(c * BPC, (c + 1) * BPC)
        in_maps.append({
            "xs": np.ascontiguousarray(xs[sl]),
            "x8": np.ascontiguousarray(x8[sl]),
            "mh": mh, "m8": m8, "gm": gm, "ident": ident,
        })

    res = run_bass_kernel_spmd(nc, in_maps, core_ids=list(range(NCORES)))
    # out per core: [C, TOK] fp16, tokens in natural order (b*N + n)
    outs = []
    for r in res.results:
        oc = r["out"].astype(np.float32).reshape(C, BPC, N)
        outs.append(oc.transpose(1, 0, 2).reshape(BPC, C, H, W))
    return np.concatenate(outs, axis=0)
